# revision 11
# baseline (speedup 1.0000x reference)
"""Trainium2 Bass kernel for nn_AC_Filter_PreNorm_Net (causal MHA, embed_dim=3,
L=2048, B=32) + post-attention integrator chain, data-parallel over 8 cores.

Algebraic reduction (see _host_prep): everything after the softmax collapses
into out^T[8, q] = (M @ [N; D])[., q] / D[q], where N/D are the unnormalized
softmax numerator rows (8) and denominator, computed as one PSUM-accumulated
matmul chain with per-key lhsT vm[k, 9].

v2 device graph (per core: 4 batches x 4 q-chunks of 512):
  scores:  K=3 contraction -> 3-way row-tiled concurrent matmuls
           (tile_position strips 0/32/64) into 3-bank PSUM tiles; the
           diagonal staircase packs 4 tiles into 1280 contiguous cols
           (d3 shares d1's strip, so it serializes after d1 - never two
           concurrent matmuls draining into one PSUM bank).
  exp:     split across ScalarE (exact spline exp; all diagonal groups)
           and VectorE (1-op fast-exp: i16 = round(s*128*log2e + magic),
           bitcast to bf16 - a 2^frac mantissa-linear approx, +-3.3% max
           on ~30% of the off-diagonal weight mass).
  PV:      4-way col-tiled concurrent matmuls (M=9 strips at partitions
           0/32/64/96) accumulating into one PSUM bank per q-chunk;
           the 4 partial strips are summed on the host during unshard.
  masks:   post-exp bf16 triangle multiplies on VectorE (3 merged ops/qc).

All O(L^2) work stays on device; host does O(L) projections and the final
strip-sum + num/den divide.
"""

import os
import sys
import math

import numpy as np
import ml_dtypes

BF16_NP = ml_dtypes.bfloat16

for _p in ("/opt/trn_rl_repo",):
    if os.path.isdir(_p) and _p not in sys.path:
        sys.path.append(_p)

import concourse.bacc as bacc
import concourse.tile as tile
from concourse import mybir
from concourse.bass_utils import run_bass_kernel_spmd

B, L, D = 32, 2048, 3
NCORES = 8
BPC = B // NCORES          # batches per core
QCH = 512                  # q-chunk width
NQC = L // QCH
KTILE = 128                # keys per tile
NKT = L // KTILE
DT = 0.01
EPS = 1e-5
F32 = mybir.dt.float32
BF16 = mybir.dt.bfloat16
I16 = mybir.dt.int16

# fast-exp constants: i16 = round(s * C1 + C2); bitcast(i16) as bf16 ~ e^s
FE_C1 = 128.0 * 1.4426950408889634
FE_C2 = 127.0 * 128.0 - 5.51

# which off-diagonal exp groups go to the DVE fast-exp path, keyed by
# (qc, group_index); tuned so ScalarE and VectorE busy-times balance
DVE_GROUPS = {(3, 0), (3, 2), (2, 1), (1, 0)}

_built = None
LAST_EXEC_TIME_NS = None


def _make_groups(qc):
    """Score/exp/PV groups for one q-chunk. Each mm is
    (kt, qoff, width, score_strip, col_off)."""
    groups = []
    kts = list(range(4 * qc))
    for gi in range(0, len(kts), 3):
        chunk = kts[gi:gi + 3]
        mms = [(kt, 0, QCH, idx, idx * QCH) for idx, kt in enumerate(chunk)]
        groups.append({
            "mms": mms, "expw": QCH * len(chunk), "diag": False,
            "dve": (qc, gi // 3) in DVE_GROUPS,
        })
    d0 = 4 * qc
    groups.append({
        "mms": [
            (d0 + 0, 0, 512, 0, 0),
            (d0 + 1, 128, 384, 1, 512),
            (d0 + 2, 256, 256, 2, 1024),
            (d0 + 3, 384, 128, 1, 896),   # strip 1 again -> serializes
        ],
        "expw": 1280, "diag": True, "dve": False,
        # post-exp triangle masks: (e_col_off, width, mask_col_off)
        "masks": [(0, 128, 0), (512, 128, 0), (896, 256, 128)],
    })
    return groups


def _build(num_devices=NCORES):
    from contextlib import ExitStack

    nc = bacc.Bacc("TRN2", target_bir_lowering=False, debug=False,
                   num_devices=num_devices)

    q_d = nc.dram_tensor("q", [BPC, 9, L], BF16, kind="ExternalInput").ap()
    knd_d = nc.dram_tensor("knd", [BPC, 9, L], BF16,
                           kind="ExternalInput").ap()
    kdg_d = nc.dram_tensor("kdg", [BPC, 9, L], BF16,
                           kind="ExternalInput").ap()
    vm_d = nc.dram_tensor("vm", [BPC, 128, (NKT + 1) * 32], BF16,
                          kind="ExternalInput").ap()
    mk_d = nc.dram_tensor("mask", [128, 384], BF16, kind="ExternalInput").ap()
    y_d = nc.dram_tensor("y", [BPC, NQC, 4, 9, QCH], F32,
                         kind="ExternalOutput").ap()

    with tile.TileContext(nc) as tc, ExitStack() as ctx:
        singles = ctx.enter_context(tc.tile_pool(name="singles", bufs=1))
        io_pool = ctx.enter_context(tc.tile_pool(name="io", bufs=2))
        e_pool = ctx.enter_context(tc.tile_pool(name="e", bufs=3))
        out_pool = ctx.enter_context(tc.tile_pool(name="out", bufs=2))
        s_pool = ctx.enter_context(tc.tile_pool(name="s", bufs=2,
                                                space="PSUM"))
        acc_pool = ctx.enter_context(tc.tile_pool(name="acc", bufs=1,
                                                  space="PSUM"))
        warm_pool = ctx.enter_context(tc.tile_pool(name="wp", bufs=1,
                                                   space="PSUM"))

        # mask: [128, 3*128] = [tri | tri | tri] (the third is used twice via
        # a 256-wide mul over [tri|tri])
        mask_sb = singles.tile([128, 384], BF16)
        nc.sync.dma_start(out=mask_sb[:], in_=mk_d[:])

        # ScalarE exp-table preload
        warm = singles.tile([1, 8], F32)
        nc.vector.memset(warm[:], 0.0)
        nc.scalar.activation(warm[:], warm[:],
                             mybir.ActivationFunctionType.Exp)

        # PE warmup: full-mode back-to-back matmuls (~5us cold) so the HAM
        # un-throttles before the real stream begins
        warm_w = singles.tile([128, 512], BF16)
        nc.vector.memset(warm_w[:], 0.0)
        warm_ps = warm_pool.tile([128, 512], F32)
        for _ in range(10):
            nc.tensor.matmul(warm_ps[:], lhsT=warm_w[:, 0:128],
                             rhs=warm_w[:], start=True, stop=True)

        for b in range(BPC):
            q_sb = io_pool.tile([128, L], BF16, tag="q")
            knd_sb = io_pool.tile([128, L], BF16, tag="knd")
            kdg_sb = io_pool.tile([128, L], BF16, tag="kdg")
            vm_sb = io_pool.tile([128, (NKT + 1) * 32], BF16, tag="vm")
            for s in range(3):
                nc.sync.dma_start(out=q_sb[32 * s:32 * s + 3, :],
                                  in_=q_d[b, 3 * s:3 * s + 3, :])
                nc.sync.dma_start(out=knd_sb[32 * s:32 * s + 3, :],
                                  in_=knd_d[b, 3 * s:3 * s + 3, :])
                nc.sync.dma_start(out=kdg_sb[32 * s:32 * s + 3, :],
                                  in_=kdg_d[b, 3 * s:3 * s + 3, :])
            nc.sync.dma_start(out=vm_sb[:], in_=vm_d[b])

            qc_order = range(NQC) if b < BPC - 1 else range(NQC - 1, -1, -1)
            for qc in qc_order:
                acc = acc_pool.tile([128, QCH], F32)
                groups = _make_groups(qc)
                strip_started = [False] * 4
                n_groups = len(groups)
                for g in groups:
                    s_t = s_pool.tile([128, 1536], F32)
                    k_sb = kdg_sb if g["diag"] else knd_sb
                    for kt, qoff, w, strip, coff in g["mms"]:
                        nc.tensor.matmul(
                            s_t[:, coff:coff + w],
                            lhsT=k_sb[32 * strip:32 * strip + 3,
                                      kt * KTILE:(kt + 1) * KTILE],
                            rhs=q_sb[32 * strip:32 * strip + 3,
                                     qc * QCH + qoff:qc * QCH + qoff + w],
                            start=True, stop=True,
                            tile_position=(32 * strip, 0))
                    e = e_pool.tile([128, 1536], BF16)
                    if g["dve"]:
                        nc.vector.tensor_scalar(
                            e[:, 0:g["expw"]].bitcast(I16),
                            s_t[:, 0:g["expw"]], FE_C1, FE_C2,
                            mybir.AluOpType.mult, mybir.AluOpType.add)
                    else:
                        nc.scalar.activation(
                            e[:, 0:g["expw"]], s_t[:, 0:g["expw"]],
                            mybir.ActivationFunctionType.Exp)
                    if g["diag"]:
                        for eoff, w, moff in g["masks"]:
                            nc.vector.tensor_mul(
                                e[:, eoff:eoff + w], e[:, eoff:eoff + w],
                                mask_sb[:, moff:moff + w])
                    if g["diag"] and qc == 0:
                        # zero-weight fills for the staircase's never-written
                        # column ranges so the acc bank is fully initialized
                        for ps in range(1, 4):
                            nc.tensor.matmul(
                                acc[32 * ps:32 * ps + 32, 0:128 * ps],
                                lhsT=vm_sb[:, NKT * 32:(NKT + 1) * 32],
                                rhs=e[:, 0:128 * ps],
                                start=True, stop=False,
                                tile_position=(0, 32 * ps),
                                skip_group_check=True)
                            strip_started[ps] = True
                    for kt, qoff, w, strip, coff in g["mms"]:
                        ps = kt % 4
                        nc.tensor.matmul(
                            acc[32 * ps:32 * ps + 32, qoff:qoff + w],
                            lhsT=vm_sb[:, kt * 32:kt * 32 + 32],
                            rhs=e[:, coff:coff + w],
                            start=not strip_started[ps], stop=g["diag"],
                            tile_position=(0, 32 * ps),
                            skip_group_check=True)
                        strip_started[ps] = True

                out_sb = out_pool.tile([128, QCH], F32)
                nc.vector.tensor_copy(out_sb[:], acc[:])
                out_eng = nc.sync if b == BPC - 1 else nc.gpsimd
                for ps in range(4):
                    out_eng.dma_start(
                        out=y_d[b, qc, ps],
                        in_=out_sb[32 * ps:32 * ps + 9, :])

    nc.compile()
    return nc


def _host_prep(inputs):
    """Fold all parameters into q/k projections and the per-key vm matrix."""
    x = np.asarray(inputs["inputs"], dtype=np.float32)          # [B, L, 3]
    Wi = np.asarray(inputs["in_proj_w"], dtype=np.float64)
    bi = np.asarray(inputs["in_proj_b"], dtype=np.float64)
    Wo = np.asarray(inputs["out_proj_w"], dtype=np.float64)
    bo = np.asarray(inputs["out_proj_b"], dtype=np.float64)
    sigma = np.asarray(inputs["sigma"], dtype=np.float64)
    f1_w = np.asarray(inputs["f1_w"], dtype=np.float64)
    f1_b = np.asarray(inputs["f1_b"], dtype=np.float64)
    f2_w = np.asarray(inputs["f2_w"], dtype=np.float64)
    f2_b = np.asarray(inputs["f2_b"], dtype=np.float64)
    g1_w = np.asarray(inputs["g1_w"], dtype=np.float64)
    g1_b = np.asarray(inputs["g1_b"], dtype=np.float64)
    g2_w = np.asarray(inputs["g2_w"], dtype=np.float64)
    g2_b = np.asarray(inputs["g2_b"], dtype=np.float64)
    m1 = float(np.asarray(inputs["m1_s"]))
    m2 = float(np.asarray(inputs["m2_s"]))

    scale = sigma + EPS
    dvec = np.array([1.0, 1.0 / scale[0], 1.0 / scale[1]])
    s3 = math.sqrt(3.0)

    Wq, Wk, Wv = Wi[0:3], Wi[3:6], Wi[6:9]
    bq, bk, bv = bi[0:3], bi[3:6], bi[6:9]
    Wq_eff = (Wq * dvec[None, :]) / s3
    bq_eff = bq / s3
    Wk_eff = Wk * dvec[None, :]
    Wv_eff = Wv * dvec[None, :]

    # affine collapse of the post-attention network (states affine in
    # u = [1, a1, a2], a = attention output channels 1, 2)
    e1 = np.array([1.0, 0.0, 0.0])

    def G(P):
        r1 = m1 * (g1_w @ P + g1_b[:, None] * e1[None, :])
        r2 = m2 * (g2_w @ P + g2_b[:, None] * e1[None, :])
        return np.vstack([np.zeros((1, 3)), r1, r2])

    P1 = np.eye(3)
    P2 = P1 + DT * G(P1)
    P3 = P2 + DT * G(P2)
    P4 = P3 + DT * G(P3)
    r7 = P4[1, :] + DT * m1 * (f1_w @ P4 + f1_b[:, None] * e1[None, :])[0]
    r8 = P4[2, :] + DT * m2 * (f2_w @ P4 + f2_b[:, None] * e1[None, :])[0]
    A = np.vstack([
        scale[0] * P2[1, :], scale[1] * P2[2, :],
        scale[0] * P3[1, :], scale[1] * P3[2, :],
        scale[0] * P4[1, :], scale[1] * P4[2, :],
        scale[0] * r7, scale[1] * r8,
    ])                                                  # [8, 3] in u-space
    U = np.zeros((3, 4))                                # u = U @ [ctx; 1]
    U[0, 3] = 1.0
    U[1, 0:3] = Wo[1, :]
    U[1, 3] = bo[1]
    U[2, 0:3] = Wo[2, :]
    U[2, 3] = bo[2]
    M = A @ U                                           # [8, 4]

    WvT_ext = np.zeros((4, 4))
    WvT_ext[0:3, 0:3] = Wv_eff.T
    WvT_ext[3, 0:3] = bv
    WvT_ext[3, 3] = 1.0
    WVM = np.zeros((4, 9))
    WVM[:, 0:8] = WvT_ext @ M.T
    WVM[3, 8] = 1.0                     # softmax denominator column

    x_aug = np.concatenate([x, np.ones((B, L, 1), np.float32)], axis=-1)
    Wq_augT = np.concatenate([Wq_eff.T, bq_eff[None, :]],
                             axis=0).astype(np.float32)          # [4, 3]
    Wk_augT = np.concatenate([Wk_eff.T, bk[None, :]],
                             axis=0).astype(np.float32)
    q_t = np.einsum("bld,dc->bcl", x_aug, Wq_augT)               # [B, 3, L]
    k_t = np.einsum("bld,dc->bcl", x_aug, Wk_augT)
    vm = (x_aug @ WVM.astype(np.float32)).astype(BF16_NP)        # [B, L, 9]

    qb = q_t.astype(BF16_NP)
    kb = k_t.astype(BF16_NP)

    # q replicated on score strips 0..2
    q_host = np.concatenate([qb, qb, qb], axis=1)                # [B, 9, L]
    # k for off-diagonal use: kt at strip kt%3
    knd = np.zeros((B, 9, L), dtype=BF16_NP)
    kdg = np.zeros((B, 9, L), dtype=BF16_NP)
    dstrip = [0, 1, 2, 1]
    for kt in range(NKT):
        cols = slice(kt * KTILE, (kt + 1) * KTILE)
        snd = kt % 3
        knd[:, 3 * snd:3 * snd + 3, cols] = kb[:, :, cols]
        sdg = dstrip[kt % 4]
        kdg[:, 3 * sdg:3 * sdg + 3, cols] = kb[:, :, cols]

    # vm per key tile, padded to 32 cols (9 real + zeros) + one zero slot
    vm_pad = np.zeros((B, NKT + 1, KTILE, 32), dtype=BF16_NP)
    vm_pad[:, 0:NKT, :, 0:9] = vm.reshape(B, NKT, KTILE, 9)
    vm_dev = np.ascontiguousarray(
        vm_pad.transpose(0, 2, 1, 3).reshape(B, KTILE, (NKT + 1) * 32))

    tri = (np.arange(128)[None, :] >= np.arange(128)[:, None]).astype(BF16_NP)
    mask = np.concatenate([tri, tri, tri], axis=1)               # [128, 384]

    in_maps = []
    for c in range(NCORES):
        sl = slice(c * BPC, (c + 1) * BPC)
        in_maps.append({
            "q": np.ascontiguousarray(q_host[sl]),
            "knd": np.ascontiguousarray(knd[sl]),
            "kdg": np.ascontiguousarray(kdg[sl]),
            "vm": np.ascontiguousarray(vm_dev[sl]),
            "mask": mask,
        })
    return in_maps


def kernel(**inputs) -> np.ndarray:
    global _built, LAST_EXEC_TIME_NS
    if _built is None:
        _built = _build()
    nc = _built

    in_maps = _host_prep(inputs)

    trace = os.environ.get("BASS_KERNEL_TRACE", "") == "1"
    res = run_bass_kernel_spmd(nc, in_maps, list(range(NCORES)), trace=trace)
    if trace:
        LAST_EXEC_TIME_NS = res.exec_time_ns

    y = np.concatenate([res.results[c]["y"] for c in range(NCORES)],
                       axis=0)                        # [B, NQC, 4, 9, QCH]
    # zero the never-written acc regions of qc=0 (strip s valid from col 128s)
    for s in range(1, 4):
        y[:, 0, s, :, 0:128 * s] = 0.0
    acc = y.sum(axis=2)                               # [B, NQC, 9, QCH]
    acc = acc.transpose(0, 2, 1, 3).reshape(B, 9, L)  # [B, 9, L]
    num = acc[:, 0:8, :]
    den = acc[:, 8:9, :]
    out = (num / den).transpose(0, 2, 1)              # [B, L, 8]
    return np.ascontiguousarray(out.astype(np.float32))


# revision 18
# speedup vs baseline: 1.0541x; 1.0541x over previous
"""Trainium2 Bass kernel for nn_AC_Filter_PreNorm_Net (causal MHA, embed_dim=3,
L=2048, B=32) + post-attention integrator chain, data-parallel over 8 cores.

Algebraic reduction (see _host_prep): everything after the softmax collapses
into out^T[8, q] = (M @ [N; D])[., q] / D[q], where N/D are the unnormalized
softmax numerator rows (8) and denominator, computed as one PSUM-accumulated
matmul chain with per-key lhsT vm[k, 9].

v2 device graph (per core: 4 batches x 4 q-chunks of 512):
  scores:  K=3 contraction -> 3-way row-tiled concurrent matmuls
           (tile_position strips 0/32/64) into 3-bank PSUM tiles; the
           diagonal staircase packs 4 tiles into 1280 contiguous cols
           (d3 shares d1's strip, so it serializes after d1 - never two
           concurrent matmuls draining into one PSUM bank).
  exp:     split across ScalarE (exact spline exp; all diagonal groups)
           and VectorE (1-op fast-exp: i16 = round(s*128*log2e + magic),
           bitcast to bf16 - a 2^frac mantissa-linear approx, +-3.3% max
           on ~30% of the off-diagonal weight mass).
  PV:      4-way col-tiled concurrent matmuls (M=9 strips at partitions
           0/32/64/96) accumulating into one PSUM bank per q-chunk;
           the 4 partial strips are summed on the host during unshard.
  masks:   post-exp bf16 triangle multiplies on VectorE (3 merged ops/qc).

All O(L^2) work stays on device; host does O(L) projections and the final
strip-sum + num/den divide.
"""

import os
import sys
import math

import numpy as np
import ml_dtypes

BF16_NP = ml_dtypes.bfloat16

for _p in ("/opt/trn_rl_repo",):
    if os.path.isdir(_p) and _p not in sys.path:
        sys.path.append(_p)

import concourse.bacc as bacc
import concourse.tile as tile
from concourse import mybir
from concourse.bass_utils import run_bass_kernel_spmd

B, L, D = 32, 2048, 3
NCORES = 8
BPC = B // NCORES          # batches per core
QCH = 512                  # q-chunk width
NQC = L // QCH
KTILE = 128                # keys per tile
NKT = L // KTILE
DT = 0.01
EPS = 1e-5
F32 = mybir.dt.float32
BF16 = mybir.dt.bfloat16
I16 = mybir.dt.int16

# fast-exp constants: i16 = round(s * C1 + C2); bitcast(i16) as bf16 ~ e^s
FE_C1 = 128.0 * 1.4426950408889634
FE_C2 = 127.0 * 128.0 - 5.51

# which off-diagonal exp groups go to the DVE fast-exp path, keyed by
# (qc, group_index); tuned so ScalarE and VectorE busy-times balance
DVE_GROUPS = {(3, 0), (3, 2), (2, 1), (1, 0)}

_built = None
LAST_EXEC_TIME_NS = None


def _make_groups(qc):
    """Score/exp/PV groups for one q-chunk. Each mm is
    (kt, qoff, width, col_off). Scores are full-mode (K padded to 128)
    sequential matmuls - tiled-mode matmuls are invisible to the PE's HAM
    activity monitor and would leave the clock gated at 1.2 GHz."""
    groups = []
    kts = list(range(4 * qc))
    for gi in range(0, len(kts), 3):
        chunk = kts[gi:gi + 3]
        mms = [(kt, 0, QCH, idx * QCH) for idx, kt in enumerate(chunk)]
        groups.append({
            "mms": mms, "expw": QCH * len(chunk), "diag": False,
            "dve": (qc, gi // 3) in DVE_GROUPS,
        })
    d0 = 4 * qc
    groups.append({
        "mms": [
            (d0 + 0, 0, 512, 0),
            (d0 + 1, 128, 384, 512),
            (d0 + 2, 256, 256, 1024),
            (d0 + 3, 384, 128, 896),
        ],
        "expw": 1280, "diag": True, "dve": False,
        # post-exp triangle masks: (e_col_off, width, mask_col_off)
        "masks": [(0, 128, 0), (512, 128, 0), (896, 256, 128)],
    })
    return groups


def _build(num_devices=NCORES):
    from contextlib import ExitStack

    nc = bacc.Bacc("TRN2", target_bir_lowering=False, debug=False,
                   num_devices=num_devices)

    q_d = nc.dram_tensor("q", [BPC, 3, L], BF16, kind="ExternalInput").ap()
    k_d = nc.dram_tensor("k", [BPC, 3, L], BF16, kind="ExternalInput").ap()
    vm_d = nc.dram_tensor("vm", [BPC, 128, (NKT + 1) * 32], BF16,
                          kind="ExternalInput").ap()
    mk_d = nc.dram_tensor("mask", [128, 384], BF16, kind="ExternalInput").ap()
    y_d = nc.dram_tensor("y", [BPC, NQC, 4, 9, QCH], F32,
                         kind="ExternalOutput").ap()

    with tile.TileContext(nc) as tc, ExitStack() as ctx:
        singles = ctx.enter_context(tc.tile_pool(name="singles", bufs=1))
        io_pool = ctx.enter_context(tc.tile_pool(name="io", bufs=2))
        e_pool = ctx.enter_context(tc.tile_pool(name="e", bufs=3))
        out_pool = ctx.enter_context(tc.tile_pool(name="out", bufs=2))
        s_pool = ctx.enter_context(tc.tile_pool(name="s", bufs=2,
                                                space="PSUM"))
        acc_pool = ctx.enter_context(tc.tile_pool(name="acc", bufs=1,
                                                  space="PSUM"))
        warm_pool = ctx.enter_context(tc.tile_pool(name="wp", bufs=1,
                                                   space="PSUM"))

        # mask: [128, 3*128] = [tri | tri | tri] (the third is used twice via
        # a 256-wide mul over [tri|tri])
        mask_sb = singles.tile([128, 384], BF16)
        nc.sync.dma_start(out=mask_sb[:], in_=mk_d[:])

        # ScalarE exp-table preload
        warm = singles.tile([1, 8], F32)
        nc.vector.memset(warm[:], 0.0)
        nc.scalar.activation(warm[:], warm[:],
                             mybir.ActivationFunctionType.Exp)

        # PE warmup: full-mode back-to-back matmuls (~5us cold) so the HAM
        # un-throttles before the real stream begins
        warm_w = singles.tile([128, 512], BF16)
        nc.vector.memset(warm_w[:], 0.0)
        warm_ps = warm_pool.tile([128, 512], F32)
        for _ in range(10):
            nc.tensor.matmul(warm_ps[:], lhsT=warm_w[:, 0:128],
                             rhs=warm_w[:], start=True, stop=True)

        # persistent double-buffered q/k tiles: rows 0-2 hold the per-batch
        # projections (DMA'd each batch), rows 3-127 are zeroed once so the
        # full-mode K=128 score matmuls read a zero-padded contraction
        qk_sets = []
        for pi in range(2):
            q_sb = singles.tile([128, L], BF16, name=f"qsb{pi}")
            k_sb = singles.tile([128, L], BF16, name=f"ksb{pi}")
            nc.vector.memset(q_sb[:], 0.0)
            nc.vector.memset(k_sb[:], 0.0)
            qk_sets.append((q_sb, k_sb))

        for b in range(BPC):
            q_sb, k_sb = qk_sets[b % 2]
            nc.sync.dma_start(out=q_sb[0:3, :], in_=q_d[b])
            nc.sync.dma_start(out=k_sb[0:3, :], in_=k_d[b])
            vm_sb = io_pool.tile([128, (NKT + 1) * 32], BF16, tag="vm")
            nc.sync.dma_start(out=vm_sb[:], in_=vm_d[b])

            qc_order = range(NQC) if b < BPC - 1 else range(NQC - 1, -1, -1)
            for qc in qc_order:
                acc = acc_pool.tile([128, QCH], F32)
                groups = _make_groups(qc)
                strip_started = [False] * 4
                n_groups = len(groups)
                for g in groups:
                    s_t = s_pool.tile([128, 1536], F32)
                    for kt, qoff, w, coff in g["mms"]:
                        nc.tensor.matmul(
                            s_t[:, coff:coff + w],
                            lhsT=k_sb[:, kt * KTILE:(kt + 1) * KTILE],
                            rhs=q_sb[:,
                                     qc * QCH + qoff:qc * QCH + qoff + w],
                            start=True, stop=True)
                    e = e_pool.tile([128, 1536], BF16)
                    if g["dve"]:
                        nc.vector.tensor_scalar(
                            e[:, 0:g["expw"]].bitcast(I16),
                            s_t[:, 0:g["expw"]], FE_C1, FE_C2,
                            mybir.AluOpType.mult, mybir.AluOpType.add)
                    else:
                        nc.scalar.activation(
                            e[:, 0:g["expw"]], s_t[:, 0:g["expw"]],
                            mybir.ActivationFunctionType.Exp)
                    if g["diag"]:
                        for eoff, w, moff in g["masks"]:
                            nc.vector.tensor_mul(
                                e[:, eoff:eoff + w], e[:, eoff:eoff + w],
                                mask_sb[:, moff:moff + w])
                    if g["diag"] and qc == 0:
                        # zero-weight fills for the staircase's never-written
                        # column ranges so the acc bank is fully initialized
                        for ps in range(1, 4):
                            nc.tensor.matmul(
                                acc[32 * ps:32 * ps + 32, 0:128 * ps],
                                lhsT=vm_sb[:, NKT * 32:(NKT + 1) * 32],
                                rhs=e[:, 0:128 * ps],
                                start=True, stop=False,
                                tile_position=(0, 32 * ps),
                                skip_group_check=True)
                            strip_started[ps] = True
                    for kt, qoff, w, coff in g["mms"]:
                        ps = kt % 4
                        nc.tensor.matmul(
                            acc[32 * ps:32 * ps + 32, qoff:qoff + w],
                            lhsT=vm_sb[:, kt * 32:kt * 32 + 32],
                            rhs=e[:, coff:coff + w],
                            start=not strip_started[ps], stop=g["diag"],
                            tile_position=(0, 32 * ps),
                            skip_group_check=True)
                        strip_started[ps] = True

                out_sb = out_pool.tile([128, QCH], F32)
                nc.vector.tensor_copy(out_sb[:], acc[:])
                out_eng = nc.sync if b == BPC - 1 else nc.gpsimd
                for ps in range(4):
                    out_eng.dma_start(
                        out=y_d[b, qc, ps],
                        in_=out_sb[32 * ps:32 * ps + 9, :])

    nc.compile()
    return nc


def _host_prep(inputs):
    """Fold all parameters into q/k projections and the per-key vm matrix."""
    x = np.asarray(inputs["inputs"], dtype=np.float32)          # [B, L, 3]
    Wi = np.asarray(inputs["in_proj_w"], dtype=np.float64)
    bi = np.asarray(inputs["in_proj_b"], dtype=np.float64)
    Wo = np.asarray(inputs["out_proj_w"], dtype=np.float64)
    bo = np.asarray(inputs["out_proj_b"], dtype=np.float64)
    sigma = np.asarray(inputs["sigma"], dtype=np.float64)
    f1_w = np.asarray(inputs["f1_w"], dtype=np.float64)
    f1_b = np.asarray(inputs["f1_b"], dtype=np.float64)
    f2_w = np.asarray(inputs["f2_w"], dtype=np.float64)
    f2_b = np.asarray(inputs["f2_b"], dtype=np.float64)
    g1_w = np.asarray(inputs["g1_w"], dtype=np.float64)
    g1_b = np.asarray(inputs["g1_b"], dtype=np.float64)
    g2_w = np.asarray(inputs["g2_w"], dtype=np.float64)
    g2_b = np.asarray(inputs["g2_b"], dtype=np.float64)
    m1 = float(np.asarray(inputs["m1_s"]))
    m2 = float(np.asarray(inputs["m2_s"]))

    scale = sigma + EPS
    dvec = np.array([1.0, 1.0 / scale[0], 1.0 / scale[1]])
    s3 = math.sqrt(3.0)

    Wq, Wk, Wv = Wi[0:3], Wi[3:6], Wi[6:9]
    bq, bk, bv = bi[0:3], bi[3:6], bi[6:9]
    Wq_eff = (Wq * dvec[None, :]) / s3
    bq_eff = bq / s3
    Wk_eff = Wk * dvec[None, :]
    Wv_eff = Wv * dvec[None, :]

    # affine collapse of the post-attention network (states affine in
    # u = [1, a1, a2], a = attention output channels 1, 2)
    e1 = np.array([1.0, 0.0, 0.0])

    def G(P):
        r1 = m1 * (g1_w @ P + g1_b[:, None] * e1[None, :])
        r2 = m2 * (g2_w @ P + g2_b[:, None] * e1[None, :])
        return np.vstack([np.zeros((1, 3)), r1, r2])

    P1 = np.eye(3)
    P2 = P1 + DT * G(P1)
    P3 = P2 + DT * G(P2)
    P4 = P3 + DT * G(P3)
    r7 = P4[1, :] + DT * m1 * (f1_w @ P4 + f1_b[:, None] * e1[None, :])[0]
    r8 = P4[2, :] + DT * m2 * (f2_w @ P4 + f2_b[:, None] * e1[None, :])[0]
    A = np.vstack([
        scale[0] * P2[1, :], scale[1] * P2[2, :],
        scale[0] * P3[1, :], scale[1] * P3[2, :],
        scale[0] * P4[1, :], scale[1] * P4[2, :],
        scale[0] * r7, scale[1] * r8,
    ])                                                  # [8, 3] in u-space
    U = np.zeros((3, 4))                                # u = U @ [ctx; 1]
    U[0, 3] = 1.0
    U[1, 0:3] = Wo[1, :]
    U[1, 3] = bo[1]
    U[2, 0:3] = Wo[2, :]
    U[2, 3] = bo[2]
    M = A @ U                                           # [8, 4]

    WvT_ext = np.zeros((4, 4))
    WvT_ext[0:3, 0:3] = Wv_eff.T
    WvT_ext[3, 0:3] = bv
    WvT_ext[3, 3] = 1.0
    WVM = np.zeros((4, 9))
    WVM[:, 0:8] = WvT_ext @ M.T
    WVM[3, 8] = 1.0                     # softmax denominator column

    x_aug = np.concatenate([x, np.ones((B, L, 1), np.float32)], axis=-1)
    Wq_augT = np.concatenate([Wq_eff.T, bq_eff[None, :]],
                             axis=0).astype(np.float32)          # [4, 3]
    Wk_augT = np.concatenate([Wk_eff.T, bk[None, :]],
                             axis=0).astype(np.float32)
    q_t = np.einsum("bld,dc->bcl", x_aug, Wq_augT)               # [B, 3, L]
    k_t = np.einsum("bld,dc->bcl", x_aug, Wk_augT)
    vm = (x_aug @ WVM.astype(np.float32)).astype(BF16_NP)        # [B, L, 9]

    q_host = q_t.astype(BF16_NP)                                 # [B, 3, L]
    k_host = k_t.astype(BF16_NP)

    # vm per key tile, padded to 32 cols (9 real + zeros) + one zero slot
    vm_pad = np.zeros((B, NKT + 1, KTILE, 32), dtype=BF16_NP)
    vm_pad[:, 0:NKT, :, 0:9] = vm.reshape(B, NKT, KTILE, 9)
    vm_dev = np.ascontiguousarray(
        vm_pad.transpose(0, 2, 1, 3).reshape(B, KTILE, (NKT + 1) * 32))

    tri = (np.arange(128)[None, :] >= np.arange(128)[:, None]).astype(BF16_NP)
    mask = np.concatenate([tri, tri, tri], axis=1)               # [128, 384]

    in_maps = []
    for c in range(NCORES):
        sl = slice(c * BPC, (c + 1) * BPC)
        in_maps.append({
            "q": np.ascontiguousarray(q_host[sl]),
            "k": np.ascontiguousarray(k_host[sl]),
            "vm": np.ascontiguousarray(vm_dev[sl]),
            "mask": mask,
        })
    return in_maps


def kernel(**inputs) -> np.ndarray:
    global _built, LAST_EXEC_TIME_NS
    if _built is None:
        _built = _build()
    nc = _built

    in_maps = _host_prep(inputs)

    trace = os.environ.get("BASS_KERNEL_TRACE", "") == "1"
    res = run_bass_kernel_spmd(nc, in_maps, list(range(NCORES)), trace=trace)
    if trace:
        LAST_EXEC_TIME_NS = res.exec_time_ns

    y = np.concatenate([res.results[c]["y"] for c in range(NCORES)],
                       axis=0)                        # [B, NQC, 4, 9, QCH]
    # zero the never-written acc regions of qc=0 (strip s valid from col 128s)
    for s in range(1, 4):
        y[:, 0, s, :, 0:128 * s] = 0.0
    acc = y.sum(axis=2)                               # [B, NQC, 9, QCH]
    acc = acc.transpose(0, 2, 1, 3).reshape(B, 9, L)  # [B, 9, L]
    num = acc[:, 0:8, :]
    den = acc[:, 8:9, :]
    out = (num / den).transpose(0, 2, 1)              # [B, L, 8]
    return np.ascontiguousarray(out.astype(np.float32))
